# revision 11
# baseline (speedup 1.0000x reference)
"""Trainium2 Bass kernel for the ACG mixture log-likelihood — layout B.

Reference (N=100000, P=256, K=64, R=16):
    D_k = I_R + M_k^T M_k;  quad_nk = ||x_n W_k||^2, W_k = M_k chol(D_k)^-T
    out = sum_n logsumexp_k [c_k - 128 ln(1 - quad_nk)]

Device pipeline per 512-sample chunk (components-on-partitions):
    Y^T[f, n] = W^T X^T      8 fp8 DoubleRow matmuls (f = r*64+k, 8 blocks)
    ysq[f, n] = (s y)^2      ACT Square->fp8 (chunk 0) / DVE custom op (chunk 1)
    q[kk, n]  = sel^T ysq    8 fp8 DoubleRow selector matmuls, PSUM-accumulated;
                             chunk 0's selector writes rows 0-63, chunk 1's
                             rows 64-127 (zero-padded columns), so chunk pairs
                             stack into one [128, 512] q tile
    lp = Ln(g - q*2^-9)      ACT with per-partition bias AP
    e  = Exp(-128 lp)        ACT -> bf16
    s  = ones^T e            1 bf16 matmul = k-sum -> [2, 512]
    host: sum ln(s) in float64 (+ N*C0).

Sharding: data-parallel over N across 8 cores; W/sel/g replicated.
Self-contained: only needs /opt/trn_rl_repo (the in-container Bass repo).
"""

import math
import os
import sys

import numpy as np

sys.path.insert(0, "/opt/trn_rl_repo")

import ml_dtypes

BF16 = ml_dtypes.bfloat16
FP8 = ml_dtypes.float8_e4m3fn

XSCALE = 64.0
WSCALE = 256.0
AB2 = float(XSCALE * WSCALE) ** 2      # (scale on y)^2 entering PSUM
YSQ_SCALE = float(2.0 ** (-19))        # ysq_fp8 = (XSCALE*WSCALE*y)^2 * 2^-19

N_TOT, P, K, R = 100000, 256, 64, 16
NCORES = 8
NSH_REAL = N_TOT // NCORES             # 12500
CH = 512                               # samples per chunk
NCHUNK = 25                            # 12800 padded samples per core
NSH = NCHUNK * CH
NCP = (NCHUNK + 1) // 2                # 13 chunk-pairs (last one single)
NB = 8                                 # f-blocks of 128 (f = r*64 + k)

_STATE: dict = {}
LAST_EXEC_NS = None


def _register_sq_op():
    """Register SQ_SCALE_ANT: out = Src0^2 * imm2 (single-stream square).
    Uses the documented custom-DVE authoring API; idempotent."""
    from concourse import dve_ops
    from concourse.dve_spec import Spec, Src0, C2, sq, lower
    from concourse.dve_uop import DveOpSpec

    for op in dve_ops.OPS:
        if op.name == "SQ_SCALE_ANT":
            return op
    spec = Spec(
        body=sq(Src0) * C2,
        reference=lambda in0, in1, c0, c1, c2: in0.astype(np.float32) ** 2 * c2,
    )
    row = dve_ops._CUSTOM_DVE_ROW_BASE + len(dve_ops.OPS)
    shas = {}
    for ver in ("v3", "v4"):
        tmp = DveOpSpec(name="SQ_SCALE_ANT", opcode=row,
                        uops=lower(spec, ver=ver), rd1_en=False)
        shas[ver] = tmp.sha(ver)
    op = dve_ops.DveOp("SQ_SCALE_ANT", spec, subdim=False, uops_sha=shas)
    dve_ops.OPS.append(op)
    dve_ops.CUSTOM_DVE_SPECS[op.name] = spec
    dve_ops._SUB_OPCODE_FOR_NAME[op.name] = row
    return op


def _fold_params(M: np.ndarray, pi: np.ndarray):
    M64 = M.astype(np.float64)
    pi64 = pi.astype(np.float64)
    D = np.eye(R)[None, :, :] + np.einsum("kpr,kps->krs", M64, M64)
    C = np.linalg.cholesky(D)
    logdet = 2.0 * np.log(np.diagonal(C, axis1=1, axis2=2)).sum(axis=1)
    Cinv = np.linalg.inv(C)
    W = np.einsum("kpr,ksr->kps", M64, Cinv)      # [K, P, R]
    mx = pi64.max()
    logpi = pi64 - (mx + math.log(np.exp(pi64 - mx).sum()))
    half_p = 0.5 * P
    logSA = math.lgamma(half_p) - math.log(2.0) - half_p * math.log(math.pi)
    c = logSA - 0.5 * logdet + logpi
    C0 = float(c.max())
    g = np.exp((C0 - c) / 128.0)                  # [K] >= 1
    Ws = W * np.sqrt(g)[:, None, None]
    Wf = np.transpose(Ws, (1, 2, 0)).reshape(P, R * K)   # f = r*64 + k
    lse_c = C0 + math.log(np.exp(c - C0).sum())
    return Wf, g, C0, lse_c


def _cast_fp8(a: np.ndarray) -> np.ndarray:
    # TRN fp8e4 saturates at +-240 (OCP e4m3fn values above map to TRN NaN)
    return np.clip(a, -240.0, 240.0).astype(FP8)


def _build(sq_op):
    import concourse.mybir as mybir
    import concourse.tile as tile
    from concourse import bacc

    f32 = mybir.dt.float32
    bf16 = mybir.dt.bfloat16
    fp8 = mybir.dt.float8e4
    DR = mybir.MatmulPerfMode.DoubleRow
    AF = mybir.ActivationFunctionType

    # Keep Square/Ln/Exp in one ACT table set so only one table load happens.
    if not _STATE.get("act_tables_patched"):
        _orig_tables = bacc.get_activation_tables

        def _patched_tables(arch):
            tabs = _orig_tables(arch)
            keep = "natural_log_exp_and_others"
            if keep in tabs:
                for name, fns in tabs.items():
                    if name != keep:
                        fns.difference_update({AF.Square, AF.Ln, AF.Exp})
            return tabs

        bacc.get_activation_tables = _patched_tables
        _STATE["act_tables_patched"] = True

    nc = bacc.Bacc("TRN2", target_bir_lowering=False, debug=False,
                   num_devices=NCORES)
    xt_d = nc.dram_tensor("xt", [2, 128, NSH], fp8, kind="ExternalInput")
    w_d = nc.dram_tensor("w", [128, 2, NB, 128], fp8, kind="ExternalInput")
    sel_d = nc.dram_tensor("sel", [128, 2, 2, 128], fp8, kind="ExternalInput")
    gb_d = nc.dram_tensor("gb", [128, 2], f32, kind="ExternalInput")
    out_d = nc.dram_tensor("out", [NCP, 128, CH], bf16, kind="ExternalOutput")

    with tile.TileContext(nc) as tc:
        with (
            tc.tile_pool(name="const", bufs=1) as cpool,
            tc.tile_pool(name="ypsum", bufs=3, space="PSUM") as ypool,
            tc.tile_pool(name="qpsum", bufs=2, space="PSUM") as qpool,
            tc.tile_pool(name="ysq", bufs=4) as spool,
            tc.tile_pool(name="post", bufs=2) as fpool,
            tc.tile_pool(name="expool", bufs=4) as epool,
        ):
            w_sb = cpool.tile([128, 2, NB, 128], fp8, tag="w")
            sel_sb = cpool.tile([128, 2, 2, 128], fp8, tag="sel")
            gb_sb = cpool.tile([128, 2], f32, tag="gb")
            nc.sync.dma_start(w_sb[:], w_d.ap())
            nc.sync.dma_start(sel_sb[:], sel_d.ap())
            nc.sync.dma_start(gb_sb[:], gb_d.ap())
            # dummy ACT so the table load runs during the DMA preamble
            warmact = cpool.tile([128, 1], bf16, tag="warmact")
            nc.scalar.activation(warmact[:], gb_sb[:, 0:1], AF.Square)

            xcs = []

            def load_x(j):
                ncols = 2 * CH if j < NCP - 1 else CH
                xc = cpool.tile([128, 2, ncols], fp8, tag=f"xt{j}")
                nsub = 2 if j == 0 else 1
                sub = ncols // nsub
                for s in range(nsub):
                    for h in range(2):
                        nc.sync.dma_start(
                            xc[:, h, s * sub:(s + 1) * sub],
                            xt_d.ap()[h, :, j * 2 * CH + s * sub:
                                      j * 2 * CH + (s + 1) * sub])
                xcs.append(xc)

            for j in range(3):
                load_x(j)

            for j in range(NCP):
                if j + 3 < NCP:
                    load_x(j + 3)
                xc = xcs[j]
                nch = 2 if j < NCP - 1 else 1
                q = qpool.tile([128, 512], f32, tag="q")
                ysqs = [[None] * 4 for _ in range(2)]

                def mains(bp):
                    yts = []
                    for ci in range(nch):
                        yt = ypool.tile([128, 2, 512], f32, tag="y")
                        for i, b in enumerate((2 * bp, 2 * bp + 1)):
                            nc.tensor.matmul(
                                yt[:, i], w_sb[:, :, b, :],
                                xc[:, :, ci * CH:(ci + 1) * CH],
                                start=True, stop=True, perf_mode=DR)
                        yts.append(yt)
                    # drains -> fp8: ACT gets chunk 0 (minus a half-tile
                    # on bp3, rebalanced to DVE); DVE gets chunk 1
                    for ci in range(nch):
                        eng = ci if nch == 2 else bp % 2
                        ysq = spool.tile([128, 2, 512], fp8, tag=f"ysq{ci}")
                        if eng == 0:
                            if bp == 3 and nch == 2:
                                nc.scalar.activation(
                                    ysq[:, :, 0:384], yts[ci][:, :, 0:384],
                                    AF.Square, scale=float(np.sqrt(YSQ_SCALE)))
                                nc.vector._custom_dve(
                                    sq_op, out=ysq[:, :, 384:512],
                                    in0=yts[ci][:, :, 384:512],
                                    imm2=YSQ_SCALE)
                            else:
                                nc.scalar.activation(
                                    ysq[:], yts[ci][:], AF.Square,
                                    scale=float(np.sqrt(YSQ_SCALE)))
                        else:
                            nc.vector._custom_dve(sq_op, out=ysq[:],
                                                  in0=yts[ci][:],
                                                  imm2=YSQ_SCALE)
                        ysqs[ci][bp] = ysq

                def sels_all():
                    for ci in range(nch):
                        for bp in range(4):
                            nc.tensor.matmul(
                                q[:], sel_sb[:, ci], ysqs[ci][bp][:],
                                start=(bp == 0 and ci == 0),
                                stop=(bp == 3 and ci == nch - 1),
                                perf_mode=DR)

                mains(0)
                mains(1)
                mains(2)
                mains(3)
                sels_all()
                lp = fpool.tile([128, 512], f32, tag="lp")
                nc.scalar.activation(lp[:], q[:], AF.Ln,
                                     scale=-1.0 / (AB2 * YSQ_SCALE),
                                     bias=gb_sb[:, 0:1])
                ex = epool.tile([128, 512], bf16, tag="ex")
                nc.scalar.activation(ex[:], lp[:], AF.Exp, scale=-128.0)
                nc.gpsimd.dma_start(out_d.ap()[j], ex[:])

    nc.compile()
    return nc


def _maybe_register_trace_hook():
    try:
        from antenv.axon_hooks import get_axon_ntff_profile_hook  # noqa: F401
        return
    except ImportError:
        pass
    import contextlib
    import ctypes
    import types

    so_path = "/opt/axon/libaxon_pjrt.so"
    if not os.path.exists(so_path):
        return
    lib = ctypes.CDLL(so_path)
    if not hasattr(lib, "axon_start_nrt_profile"):
        return
    lib.axon_start_nrt_profile.argtypes = [ctypes.POINTER(ctypes.c_int64),
                                           ctypes.c_size_t]
    lib.axon_start_nrt_profile.restype = ctypes.c_int64
    lib.axon_stop_nrt_profile.argtypes = [ctypes.c_char_p]
    lib.axon_stop_nrt_profile.restype = ctypes.c_int64

    @contextlib.contextmanager
    def _hook(output_dir, device_ids):
        import jax
        jax.devices()
        if device_ids:
            ids = (ctypes.c_int64 * len(device_ids))(*device_ids)
            rc = lib.axon_start_nrt_profile(ids, len(device_ids))
        else:
            rc = lib.axon_start_nrt_profile(None, 0)
        if rc != 0:
            raise RuntimeError(f"axon_start_nrt_profile rc={rc}")
        try:
            yield
        finally:
            n = lib.axon_stop_nrt_profile(str(output_dir).encode())
            print(f"ntff profile: {n} file(s) -> {output_dir}", file=sys.stderr)

    mod = types.ModuleType("antenv.axon_hooks")
    mod.get_axon_ntff_profile_hook = lambda: _hook
    mod.set_axon_ntff_profile_hook = lambda h: None
    sys.modules["antenv.axon_hooks"] = mod


def kernel(X: np.ndarray, M: np.ndarray, pi: np.ndarray) -> np.ndarray:
    global LAST_EXEC_NS
    from concourse.bass_utils import run_bass_kernel_spmd

    sq_op = _register_sq_op()
    if "nc" not in _STATE:
        _STATE["nc"] = _build(sq_op)
    nc = _STATE["nc"]

    Wf, g, C0, lse_c = _fold_params(M, pi)
    # w[p, i, b, c] = Wf[i*128 + p, 128*b + c] * WSCALE
    w_host = np.ascontiguousarray(
        _cast_fp8(Wf * WSCALE).reshape(2, 128, NB, 128).transpose(1, 0, 2, 3))
    # augmented selectors: sel[ci][p, i, c]: chunk 0 -> q rows 0-63,
    # chunk 1 -> q rows 64-127 (other 64 columns are zero)
    sel = np.zeros((128, 2, 2, 128), dtype=np.float32)
    for p in range(128):
        sel[p, 0, :, p % K] = 1.0
        sel[p, 1, :, K + p % K] = 1.0
    sel_host = sel.astype(FP8)
    gb_host = np.ascontiguousarray(
        np.stack([np.concatenate([g, g]).astype(np.float32)] * 2, axis=1))

    in_maps = []
    for cix in range(NCORES):
        xpad = np.zeros((NSH, P), dtype=FP8)
        xpad[:NSH_REAL] = _cast_fp8(
            X[cix * NSH_REAL:(cix + 1) * NSH_REAL] * XSCALE)
        xt = np.ascontiguousarray(xpad.T).reshape(2, 128, NSH)
        in_maps.append({"xt": xt, "w": w_host, "sel": sel_host,
                        "gb": gb_host})

    trace = bool(int(os.environ.get("KERNEL_TRACE", "0")))
    if trace:
        _maybe_register_trace_hook()
    res = run_bass_kernel_spmd(nc, in_maps, core_ids=list(range(NCORES)),
                               trace=trace)
    LAST_EXEC_NS = res.exec_time_ns
    if trace and res.exec_time_ns is not None:
        print(f"HW exec time: {res.exec_time_ns} ns")
        if res.instructions_and_trace is not None:
            print(f"trace: {res.instructions_and_trace[1]}")

    total = 0.0
    for r in res.results:
        e = r["out"].astype(np.float64)           # [NCP, 128, CH] exp terms
        s0 = e[:, 0:64, :].sum(axis=1)            # chunk 2j k-sums
        s1 = e[:NCP - 1, 64:128, :].sum(axis=1)   # chunk 2j+1 (first 12 cps)
        total += np.log(s0).sum() + np.log(s1).sum()
    n_pad = NSH * NCORES - N_TOT
    ans = total + N_TOT * C0 - n_pad * (lse_c - C0)
    return np.asarray(ans, dtype=np.float32)
